# revision 4
# baseline (speedup 1.0000x reference)
"""Trainium2 Bass kernel: CNN encoder (conv1d F=8, D=128 -> K=256, valid, + bias + ReLU).

Computation: out[b, l, k] = relu(b_k[k] + sum_{f,d} x[b, l+f, d] * filt[f,d] * W[f*D+d, k])
for l in [0, L-F)  (2040 windows).

Strategy (v3):
  - Data-parallel: 32 batches / 8 cores = 4 batches per core. Params replicated.
  - Host folds filt into W (wp[f,d,k] = filt[f,d]*W[f*128+d,k]) and transposes x
    to d-major (xT[b, d, l]) so the contraction dim (d=128) lands on SBUF
    partitions with fully-contiguous DMA.
  - bf16 weights + bf16 activations + bf16 output (host upcasts): identical PE
    throughput to f32r (1 col/cycle) but half the HBM traffic and SBUF
    footprint. Measured end-to-end quantization error 3.4e-3 (gate: 2e-2).
  - Per (batch, k-half): 4 PSUM banks hold four 512-wide output stripes; for
    each stripe the 8 filter taps accumulate via matmuls
        psum[k=128p, l<=512] += wp[kh,:,f*128:(f+1)*128].T @ xT[:, l0+f : ...]
  - Eviction fuses bias-add + ReLU (ScalarE activation / VectorE tensor_scalar
    alternating), one merged output DMA per (batch, k-half).
  - Output written k-major ([b, kh, 128, l]); host transposes back to [b, l, k].

Measured (interleaved For_i loop-difference, drift-corrected): 80.9 us per
iteration per core vs 85.4 us for the f32r baseline under the same
methodology. The PE matmul stream is the floor: 256 MMs x ~330 ns (213 ns
moving stream + ~120 ns per-MM weight-load serialization that walrus cannot
hide on TRN2; ldw-opt rejects the IR, self-loading MMs measure identically,
and identical consecutive weight APs are still reloaded). fp8e4 DoubleRow
tap-fusion was faster (-6.8 us) but its quantization error (2.1e-2) exceeds
the 2e-2 gate, so it is not used.
"""

import numpy as np

import concourse.bacc as bacc
import concourse.bass as bass
import concourse.tile as tile
import concourse.mybir as mybir
from concourse.bass_utils import run_bass_kernel_spmd

F32 = mybir.dt.float32
BF16 = mybir.dt.bfloat16

N_CORES = 8
B, L, D = 32, 2048, 128
F, K = 8, 256
N_WIN = L - F            # 2040
BP = B // N_CORES        # batches per core
KH = K // 128            # k halves
SUPERS = [(0, 512), (512, 512), (1024, 512), (1536, N_WIN - 1536)]

W_DT = BF16
X_DT = BF16
OUT_DT = BF16


def _build_program(reps=1, loop_n=0):
    """One SPMD program for all 8 cores. loop_n>0 wraps the body in a
    hardware For_i loop (benchmarking only: every iteration rewrites the
    same output region)."""
    nc = bacc.Bacc(
        "TRN2",
        target_bir_lowering=False,
        debug=False,
        num_devices=N_CORES,
    )
    xT_d = nc.declare_dram_parameter("xT", [BP, D, L], X_DT, isOutput=False)
    wp_d = nc.declare_dram_parameter("wp", [KH, D, F * 128], W_DT, isOutput=False)
    bias_d = nc.declare_dram_parameter("bias", [128, KH], F32, isOutput=False)
    out_d = nc.declare_dram_parameter(
        "outT", [reps * BP, KH, 128, N_WIN], OUT_DT, isOutput=True)

    def body(nc, tc, pools, r):
        const_pool, xt_pool, psum_pool, out_pool = pools
        bias_sb = const_pool.tile([128, KH], F32, tag="bias")
        wp_sb = [const_pool.tile([D, F * 128], W_DT, tag=f"wp{kh}",
                                 name=f"wp_sb{kh}")
                 for kh in range(KH)]
        xt = [xt_pool.tile([D, L], X_DT, tag="xt", name=f"xt{b}")
              for b in range(BP)]

        # weights + batch 0 first so compute can start ASAP
        nc.sync.dma_start(wp_sb[0][:], wp_d[0])
        nc.sync.dma_start(xt[0][:], xT_d[0])
        nc.sync.dma_start(bias_sb[:], bias_d[:])
        nc.sync.dma_start(wp_sb[1][:], wp_d[1])
        for b in range(1, BP):
            nc.sync.dma_start(xt[b][:], xT_d[b])

        evictor = 0
        for b in range(BP):
            for kh in range(KH):
                ob = out_pool.tile([128, N_WIN], OUT_DT, tag="ob", name="ob")
                pss = [psum_pool.tile([128, 512], F32, tag="ps",
                                      name=f"ps{b}_{kh}_{si}")
                       for si in range(4)]

                for si in range(4):
                    l0, ls = SUPERS[si]
                    for f in range(F):
                        nc.tensor.matmul(
                            pss[si][:, :ls],
                            lhsT=wp_sb[kh][:, f * 128:(f + 1) * 128],
                            rhs=xt[b][:, l0 + f:l0 + f + ls],
                            start=(f == 0),
                            stop=(f == F - 1),
                        )

                for si in range(4):
                    l0, ls = SUPERS[si]
                    if evictor == 0:
                        nc.scalar.activation(
                            ob[:, l0:l0 + ls], pss[si][:, :ls],
                            mybir.ActivationFunctionType.Relu,
                            bias=bias_sb[:, kh:kh + 1], scale=1.0,
                        )
                    else:
                        nc.vector.tensor_scalar(
                            ob[:, l0:l0 + ls], pss[si][:, :ls],
                            scalar1=bias_sb[:, kh:kh + 1], scalar2=0.0,
                            op0=mybir.AluOpType.add, op1=mybir.AluOpType.max,
                        )
                    evictor ^= 1
                nc.sync.dma_start(out_d[r * BP + b, kh], ob[:])

    with tile.TileContext(nc) as tc:
        with (
            tc.tile_pool(name="const", bufs=2) as const_pool,
            tc.tile_pool(name="xt", bufs=BP) as xt_pool,
            tc.tile_pool(name="psum", bufs=8, space=bass.MemorySpace.PSUM) as psum_pool,
            tc.tile_pool(name="out", bufs=4) as out_pool,
        ):
            pools = (const_pool, xt_pool, psum_pool, out_pool)
            if loop_n > 0:
                with tc.For_i(0, loop_n, 1,
                              hint_engines=(mybir.EngineType.PE,)):
                    for r in range(reps):
                        body(nc, tc, pools, r)
            else:
                for r in range(reps):
                    body(nc, tc, pools, r)
    nc.compile()
    return nc


def _prep_inputs(user_batch, filt, W_k, b_k):
    import ml_dtypes
    user_batch = np.asarray(user_batch, dtype=np.float32)
    filt = np.asarray(filt, dtype=np.float32)
    W_k = np.asarray(W_k, dtype=np.float32)
    b_k = np.asarray(b_k, dtype=np.float32)

    wp = W_k.reshape(F, D, K) * filt[:, :, None]              # [f, d, k]
    wp_host = np.ascontiguousarray(                           # [kh, d, f*128]
        wp.reshape(F, D, KH, 128).transpose(2, 1, 0, 3)).reshape(KH, D, F * 128)
    bias_host = np.ascontiguousarray(b_k.reshape(KH, 128).T)  # [128, kh]
    xT = np.ascontiguousarray(user_batch.transpose(0, 2, 1))  # [b, d, l]
    wp_host = wp_host.astype(ml_dtypes.bfloat16)
    xT = xT.astype(ml_dtypes.bfloat16)
    return xT, wp_host, bias_host


def _make_in_maps(xT, wp_host, bias_host):
    return [
        {"xT": xT[c * BP:(c + 1) * BP], "wp": wp_host, "bias": bias_host}
        for c in range(N_CORES)
    ]


def _run(user_batch, filt, W_k, b_k, trace=False):
    xT, wp_host, bias_host = _prep_inputs(user_batch, filt, W_k, b_k)
    nc = _build_program()
    in_maps = _make_in_maps(xT, wp_host, bias_host)
    res = run_bass_kernel_spmd(nc, in_maps, list(range(N_CORES)), trace=trace)
    outT = np.concatenate([np.asarray(r["outT"], dtype=np.float32)
                           for r in res.results], axis=0)  # [B, KH, 128, N_WIN]
    out = outT.reshape(B, K, N_WIN).transpose(0, 2, 1)     # [B, N_WIN, K]
    return np.ascontiguousarray(out), res


def kernel(user_batch, filt, W_k, b_k):
    out, _ = _run(user_batch, filt, W_k, b_k, trace=False)
    return out


# revision 6
# speedup vs baseline: 1.1855x; 1.1855x over previous
"""Trainium2 Bass kernel: CNN encoder (conv1d F=8, D=128 -> K=256, valid, + bias + ReLU).

Computation: out[b, l, k] = relu(b_k[k] + sum_{f,d} x[b, l+f, d] * filt[f,d] * W[f*D+d, k])
for l in [0, L-F)  (2040 windows).

Strategy (v3):
  - Data-parallel: 32 batches / 8 cores = 4 batches per core. Params replicated.
  - Host folds filt into W (wp[f,d,k] = filt[f,d]*W[f*128+d,k]) and transposes x
    to d-major (xT[b, d, l]) so the contraction dim (d=128) lands on SBUF
    partitions with fully-contiguous DMA.
  - bf16 weights + bf16 activations + bf16 output (host upcasts): identical PE
    throughput to f32r (1 col/cycle) but half the HBM traffic and SBUF
    footprint. Measured end-to-end quantization error 3.4e-3 (gate: 2e-2).
  - Per (batch, k-half): 4 PSUM banks hold four 512-wide output stripes; for
    each stripe the 8 filter taps accumulate via matmuls
        psum[k=128p, l<=512] += wp[kh,:,f*128:(f+1)*128].T @ xT[:, l0+f : ...]
  - Eviction fuses bias-add + ReLU (ScalarE activation / VectorE tensor_scalar
    alternating), one merged output DMA per (batch, k-half).
  - Output written k-major ([b, kh, 128, l]); host transposes back to [b, l, k].

Measured (interleaved For_i loop-difference, drift-corrected): 80.9 us per
iteration per core vs 85.4 us for the f32r baseline under the same
methodology. The PE matmul stream is the floor: 256 MMs x ~330 ns (213 ns
moving stream + ~120 ns per-MM weight-load serialization that walrus cannot
hide on TRN2; ldw-opt rejects the IR, self-loading MMs measure identically,
and identical consecutive weight APs are still reloaded). fp8e4 DoubleRow
tap-fusion was faster (-6.8 us) but its quantization error (2.1e-2) exceeds
the 2e-2 gate, so it is not used.
"""

import numpy as np

import concourse.bacc as bacc
import concourse.bass as bass
import concourse.tile as tile
import concourse.mybir as mybir
from concourse.bass_utils import run_bass_kernel_spmd

F32 = mybir.dt.float32
BF16 = mybir.dt.bfloat16

N_CORES = 8
B, L, D = 32, 2048, 128
F, K = 8, 256
N_WIN = L - F            # 2040
BP = B // N_CORES        # batches per core
KH = K // 128            # k halves
SUPERS = [(0, 512), (512, 512), (1024, 512), (1536, N_WIN - 1536)]

W_DT = BF16
X_DT = BF16
OUT_DT = BF16


def _build_program(reps=1, loop_n=0):
    """One SPMD program for all 8 cores. loop_n>0 wraps the body in a
    hardware For_i loop (benchmarking only: every iteration rewrites the
    same output region)."""
    nc = bacc.Bacc(
        "TRN2",
        target_bir_lowering=False,
        debug=False,
        num_devices=N_CORES,
    )
    xT_d = nc.declare_dram_parameter("xT", [BP, D, L], X_DT, isOutput=False)
    wp_d = nc.declare_dram_parameter("wp", [KH, D, F * 128], W_DT, isOutput=False)
    bias_d = nc.declare_dram_parameter("bias", [128, KH], F32, isOutput=False)
    out_d = nc.declare_dram_parameter(
        "outT", [reps * BP, KH, 128, N_WIN], OUT_DT, isOutput=True)

    def body(nc, tc, pools, r):
        const_pool, xt_pool, psum_pool, out_pool = pools
        bias_sb = const_pool.tile([128, KH], F32, tag="bias")
        wp_sb = [const_pool.tile([D, F * 128], W_DT, tag=f"wp{kh}",
                                 name=f"wp_sb{kh}")
                 for kh in range(KH)]
        xt = [xt_pool.tile([D, L], X_DT, tag="xt", name=f"xt{b}")
              for b in range(BP)]

        # Inputs go through the ACT HWDGE queue, outputs through the SP
        # queue: two independent hardware DMA queues, so next-iteration
        # input prefetch is not serialized behind output drains.
        # Weights + batch 0 first so compute can start ASAP.
        nc.scalar.dma_start(wp_sb[0][:], wp_d[0])
        nc.scalar.dma_start(xt[0][:], xT_d[0])
        nc.scalar.dma_start(bias_sb[:], bias_d[:])
        nc.scalar.dma_start(wp_sb[1][:], wp_d[1])
        for b in range(1, BP):
            nc.scalar.dma_start(xt[b][:], xT_d[b])

        evictor = 0
        for b in range(BP):
            for kh in range(KH):
                ob = out_pool.tile([128, N_WIN], OUT_DT, tag="ob", name="ob")
                pss = [psum_pool.tile([128, 512], F32, tag="ps",
                                      name=f"ps{b}_{kh}_{si}")
                       for si in range(4)]

                for si in range(4):
                    l0, ls = SUPERS[si]
                    for f in range(F):
                        nc.tensor.matmul(
                            pss[si][:, :ls],
                            lhsT=wp_sb[kh][:, f * 128:(f + 1) * 128],
                            rhs=xt[b][:, l0 + f:l0 + f + ls],
                            start=(f == 0),
                            stop=(f == F - 1),
                        )

                # last group streams per-stripe so the tail is one small DMA
                # after the final eviction instead of one big one
                last = (b == BP - 1 and kh == KH - 1)
                for si in range(4):
                    l0, ls = SUPERS[si]
                    if evictor == 0:
                        nc.scalar.activation(
                            ob[:, l0:l0 + ls], pss[si][:, :ls],
                            mybir.ActivationFunctionType.Relu,
                            bias=bias_sb[:, kh:kh + 1], scale=1.0,
                        )
                    else:
                        nc.vector.tensor_scalar(
                            ob[:, l0:l0 + ls], pss[si][:, :ls],
                            scalar1=bias_sb[:, kh:kh + 1], scalar2=0.0,
                            op0=mybir.AluOpType.add, op1=mybir.AluOpType.max,
                        )
                    evictor ^= 1
                    if last:
                        nc.sync.dma_start(
                            out_d[r * BP + b, kh, :, l0:l0 + ls],
                            ob[:, l0:l0 + ls])
                if not last:
                    nc.sync.dma_start(out_d[r * BP + b, kh], ob[:])

    with tile.TileContext(nc) as tc:
        with (
            tc.tile_pool(name="const", bufs=2) as const_pool,
            tc.tile_pool(name="xt", bufs=BP) as xt_pool,
            tc.tile_pool(name="psum", bufs=8, space=bass.MemorySpace.PSUM) as psum_pool,
            tc.tile_pool(name="out", bufs=4) as out_pool,
        ):
            pools = (const_pool, xt_pool, psum_pool, out_pool)
            if loop_n > 0:
                with tc.For_i(0, loop_n, 1,
                              hint_engines=(mybir.EngineType.PE,)):
                    for r in range(reps):
                        body(nc, tc, pools, r)
            else:
                for r in range(reps):
                    body(nc, tc, pools, r)
    nc.compile()
    return nc


def _prep_inputs(user_batch, filt, W_k, b_k):
    import ml_dtypes
    user_batch = np.asarray(user_batch, dtype=np.float32)
    filt = np.asarray(filt, dtype=np.float32)
    W_k = np.asarray(W_k, dtype=np.float32)
    b_k = np.asarray(b_k, dtype=np.float32)

    wp = W_k.reshape(F, D, K) * filt[:, :, None]              # [f, d, k]
    wp_host = np.ascontiguousarray(                           # [kh, d, f*128]
        wp.reshape(F, D, KH, 128).transpose(2, 1, 0, 3)).reshape(KH, D, F * 128)
    bias_host = np.ascontiguousarray(b_k.reshape(KH, 128).T)  # [128, kh]
    xT = np.ascontiguousarray(user_batch.transpose(0, 2, 1))  # [b, d, l]
    wp_host = wp_host.astype(ml_dtypes.bfloat16)
    xT = xT.astype(ml_dtypes.bfloat16)
    return xT, wp_host, bias_host


def _make_in_maps(xT, wp_host, bias_host):
    return [
        {"xT": xT[c * BP:(c + 1) * BP], "wp": wp_host, "bias": bias_host}
        for c in range(N_CORES)
    ]


def _run(user_batch, filt, W_k, b_k, trace=False):
    xT, wp_host, bias_host = _prep_inputs(user_batch, filt, W_k, b_k)
    nc = _build_program()
    in_maps = _make_in_maps(xT, wp_host, bias_host)
    res = run_bass_kernel_spmd(nc, in_maps, list(range(N_CORES)), trace=trace)
    outT = np.concatenate([np.asarray(r["outT"], dtype=np.float32)
                           for r in res.results], axis=0)  # [B, KH, 128, N_WIN]
    out = outT.reshape(B, K, N_WIN).transpose(0, 2, 1)     # [B, N_WIN, K]
    return np.ascontiguousarray(out), res


def kernel(user_batch, filt, W_k, b_k):
    out, _ = _run(user_batch, filt, W_k, b_k, trace=False)
    return out


# revision 8
# speedup vs baseline: 1.2003x; 1.0124x over previous
"""Trainium2 Bass kernel: CNN encoder (conv1d F=8, D=128 -> K=256, valid, + bias + ReLU).

Computation: out[b, l, k] = relu(b_k[k] + sum_{f,d} x[b, l+f, d] * filt[f,d] * W[f*D+d, k])
for l in [0, L-F)  (2040 windows).

Strategy (v3):
  - Data-parallel: 32 batches / 8 cores = 4 batches per core. Params replicated.
  - Host folds filt into W (wp[f,d,k] = filt[f,d]*W[f*128+d,k]) and transposes x
    to d-major (xT[b, d, l]) so the contraction dim (d=128) lands on SBUF
    partitions with fully-contiguous DMA.
  - bf16 weights + bf16 activations + bf16 output (host upcasts): identical PE
    throughput to f32r (1 col/cycle) but half the HBM traffic and SBUF
    footprint. Measured end-to-end quantization error 3.4e-3 (gate: 2e-2).
  - Per (batch, k-half): 4 PSUM banks hold four 512-wide output stripes; for
    each stripe the 8 filter taps accumulate via matmuls
        psum[k=128p, l<=512] += wp[kh,:,f*128:(f+1)*128].T @ xT[:, l0+f : ...]
  - Eviction fuses bias-add + ReLU (ScalarE activation / VectorE tensor_scalar
    alternating), one merged output DMA per (batch, k-half).
  - Output written k-major ([b, kh, 128, l]); host transposes back to [b, l, k].

Measured (interleaved For_i loop-difference, drift-corrected): 78.9 us per
iteration per core vs 85.4 us for the f32r baseline under the same
methodology. Input DMAs ride the ACT HWDGE queue and output DMAs the SP
queue so next-iteration input prefetch is not serialized behind output
drains (-1.3 us), and the final group's output streams per-stripe to
shorten the tail. The PE matmul stream is the floor: 256 MMs x ~330 ns (213 ns
moving stream + ~120 ns per-MM weight-load serialization that walrus cannot
hide on TRN2; ldw-opt rejects the IR, self-loading MMs measure identically,
and identical consecutive weight APs are still reloaded). fp8e4 DoubleRow
tap-fusion was faster (-6.8 us) but its quantization error (2.1e-2) exceeds
the 2e-2 gate, so it is not used.
"""

import numpy as np

import concourse.bacc as bacc
import concourse.bass as bass
import concourse.tile as tile
import concourse.mybir as mybir
from concourse.bass_utils import run_bass_kernel_spmd

F32 = mybir.dt.float32
BF16 = mybir.dt.bfloat16

N_CORES = 8
B, L, D = 32, 2048, 128
F, K = 8, 256
N_WIN = L - F            # 2040
BP = B // N_CORES        # batches per core
KH = K // 128            # k halves
SUPERS = [(0, 512), (512, 512), (1024, 512), (1536, N_WIN - 1536)]

W_DT = BF16
X_DT = BF16
OUT_DT = BF16


def _selfload_matmuls(nc):
    """Make matmuls self-loading and drop the standalone InstLdweights the
    bass pipeline pairs with them (~254 no-op instructions off the PE queue).
    LDW semaphore waits move onto the paired matmul when it has none, or are
    dropped when implied by a later same-queue DMA wait already on the
    matmul (HWDGE completions within one queue are in-order)."""
    import re

    def qidx(w):
        m = re.match(r"DMAHW(\d+)_", w.ant_name or "")
        return int(m.group(1)) if m else None

    for fn in nc.m.functions:
        for blk in fn.blocks:
            insts = blk.instructions
            keep = []
            for i, x in enumerate(insts):
                tn = type(x).__name__
                if tn == "InstMatmult":
                    x.ldweights = True
                    keep.append(x)
                    continue
                if tn != "InstLdweights":
                    keep.append(x)
                    continue
                si = x.sync_info
                waits = list(si.on_wait) if si else []
                upds = list(si.on_update) if si else []
                if upds:
                    keep.append(x)
                    continue
                if not waits:
                    continue
                nxt = insts[i + 1] if i + 1 < len(insts) else None
                if nxt is None or type(nxt).__name__ != "InstMatmult":
                    keep.append(x)
                    continue
                nsi = nxt.sync_info
                nwaits = list(nsi.on_wait) if nsi else []
                if not nwaits:
                    if nsi is None:
                        nxt.sync_info = mybir.SyncInfo(on_wait=waits,
                                                       on_update=[])
                    else:
                        nsi.on_wait = waits
                    continue
                widx = [qidx(w) for w in waits]
                nidx = [qidx(w) for w in nwaits]
                if (all(v is not None for v in widx)
                        and any(v is not None for v in nidx)
                        and max(widx) < max(v for v in nidx if v is not None)):
                    continue
                keep.append(x)
            blk.instructions[:] = keep


def _build_program(reps=1, loop_n=0):
    """One SPMD program for all 8 cores. loop_n>0 wraps the body in a
    hardware For_i loop (benchmarking only: every iteration rewrites the
    same output region)."""
    nc = bacc.Bacc(
        "TRN2",
        target_bir_lowering=False,
        debug=False,
        num_devices=N_CORES,
    )
    xT_d = nc.declare_dram_parameter("xT", [BP, D, L], X_DT, isOutput=False)
    wp_d = nc.declare_dram_parameter("wp", [KH, D, F * 128], W_DT, isOutput=False)
    bias_d = nc.declare_dram_parameter("bias", [128, KH], F32, isOutput=False)
    out_d = nc.declare_dram_parameter(
        "outT", [reps * BP, KH, 128, N_WIN], OUT_DT, isOutput=True)

    def body(nc, tc, pools, r):
        const_pool, xt_pool, psum_pool, out_pool = pools
        bias_sb = const_pool.tile([128, KH], F32, tag="bias")
        wp_sb = [const_pool.tile([D, F * 128], W_DT, tag=f"wp{kh}",
                                 name=f"wp_sb{kh}")
                 for kh in range(KH)]
        xt = [xt_pool.tile([D, L], X_DT, tag="xt", name=f"xt{b}")
              for b in range(BP)]

        # Inputs go through the ACT HWDGE queue, outputs through the SP
        # queue: two independent hardware DMA queues, so next-iteration
        # input prefetch is not serialized behind output drains.
        # Weights + batch 0 first so compute can start ASAP.
        nc.scalar.dma_start(wp_sb[0][:], wp_d[0])
        nc.scalar.dma_start(xt[0][:], xT_d[0])
        nc.scalar.dma_start(bias_sb[:], bias_d[:])
        nc.scalar.dma_start(wp_sb[1][:], wp_d[1])
        for b in range(1, BP):
            nc.scalar.dma_start(xt[b][:], xT_d[b])

        evictor = 0
        for b in range(BP):
            for kh in range(KH):
                ob = out_pool.tile([128, N_WIN], OUT_DT, tag="ob", name="ob")
                pss = [psum_pool.tile([128, 512], F32, tag="ps",
                                      name=f"ps{b}_{kh}_{si}")
                       for si in range(4)]

                for si in range(4):
                    l0, ls = SUPERS[si]
                    for f in range(F):
                        nc.tensor.matmul(
                            pss[si][:, :ls],
                            lhsT=wp_sb[kh][:, f * 128:(f + 1) * 128],
                            rhs=xt[b][:, l0 + f:l0 + f + ls],
                            start=(f == 0),
                            stop=(f == F - 1),
                        )

                # last group streams per-stripe so the tail is one small DMA
                # after the final eviction instead of one big one
                last = (b == BP - 1 and kh == KH - 1)
                for si in range(4):
                    l0, ls = SUPERS[si]
                    if evictor == 0:
                        nc.scalar.activation(
                            ob[:, l0:l0 + ls], pss[si][:, :ls],
                            mybir.ActivationFunctionType.Relu,
                            bias=bias_sb[:, kh:kh + 1], scale=1.0,
                        )
                    else:
                        nc.vector.tensor_scalar(
                            ob[:, l0:l0 + ls], pss[si][:, :ls],
                            scalar1=bias_sb[:, kh:kh + 1], scalar2=0.0,
                            op0=mybir.AluOpType.add, op1=mybir.AluOpType.max,
                        )
                    evictor ^= 1
                    if last:
                        nc.sync.dma_start(
                            out_d[r * BP + b, kh, :, l0:l0 + ls],
                            ob[:, l0:l0 + ls])
                if not last:
                    nc.sync.dma_start(out_d[r * BP + b, kh], ob[:])

    with tile.TileContext(nc) as tc:
        with (
            tc.tile_pool(name="const", bufs=2) as const_pool,
            tc.tile_pool(name="xt", bufs=BP) as xt_pool,
            tc.tile_pool(name="psum", bufs=8, space=bass.MemorySpace.PSUM) as psum_pool,
            tc.tile_pool(name="out", bufs=4) as out_pool,
        ):
            pools = (const_pool, xt_pool, psum_pool, out_pool)
            if loop_n > 0:
                with tc.For_i(0, loop_n, 1,
                              hint_engines=(mybir.EngineType.PE,)):
                    for r in range(reps):
                        body(nc, tc, pools, r)
            else:
                for r in range(reps):
                    body(nc, tc, pools, r)
    nc.compile()
    _selfload_matmuls(nc)
    return nc


def _prep_inputs(user_batch, filt, W_k, b_k):
    import ml_dtypes
    user_batch = np.asarray(user_batch, dtype=np.float32)
    filt = np.asarray(filt, dtype=np.float32)
    W_k = np.asarray(W_k, dtype=np.float32)
    b_k = np.asarray(b_k, dtype=np.float32)

    wp = W_k.reshape(F, D, K) * filt[:, :, None]              # [f, d, k]
    wp_host = np.ascontiguousarray(                           # [kh, d, f*128]
        wp.reshape(F, D, KH, 128).transpose(2, 1, 0, 3)).reshape(KH, D, F * 128)
    bias_host = np.ascontiguousarray(b_k.reshape(KH, 128).T)  # [128, kh]
    xT = np.ascontiguousarray(user_batch.transpose(0, 2, 1))  # [b, d, l]
    wp_host = wp_host.astype(ml_dtypes.bfloat16)
    xT = xT.astype(ml_dtypes.bfloat16)
    return xT, wp_host, bias_host


def _make_in_maps(xT, wp_host, bias_host):
    return [
        {"xT": xT[c * BP:(c + 1) * BP], "wp": wp_host, "bias": bias_host}
        for c in range(N_CORES)
    ]


def _run(user_batch, filt, W_k, b_k, trace=False):
    xT, wp_host, bias_host = _prep_inputs(user_batch, filt, W_k, b_k)
    nc = _build_program()
    in_maps = _make_in_maps(xT, wp_host, bias_host)
    res = run_bass_kernel_spmd(nc, in_maps, list(range(N_CORES)), trace=trace)
    outT = np.concatenate([np.asarray(r["outT"], dtype=np.float32)
                           for r in res.results], axis=0)  # [B, KH, 128, N_WIN]
    out = outT.reshape(B, K, N_WIN).transpose(0, 2, 1)     # [B, N_WIN, K]
    return np.ascontiguousarray(out), res


def kernel(user_batch, filt, W_k, b_k):
    out, _ = _run(user_batch, filt, W_k, b_k, trace=False)
    return out


# revision 10
# speedup vs baseline: 1.2300x; 1.0248x over previous
"""Trainium2 Bass kernel: CNN encoder (conv1d F=8, D=128 -> K=256, valid, + bias + ReLU).

Computation: out[b, l, k] = relu(b_k[k] + sum_{f,d} x[b, l+f, d] * filt[f,d] * W[f*D+d, k])
for l in [0, L-F)  (2040 windows).

Strategy (v3):
  - Data-parallel: 32 batches / 8 cores = 4 batches per core. Params replicated.
  - Host folds filt into W (wp[f,d,k] = filt[f,d]*W[f*128+d,k]) and transposes x
    to d-major (xT[b, d, l]) so the contraction dim (d=128) lands on SBUF
    partitions with fully-contiguous DMA.
  - bf16 weights + bf16 activations + bf16 output (host upcasts): identical PE
    throughput to f32r (1 col/cycle) but half the HBM traffic and SBUF
    footprint. Measured end-to-end quantization error 3.4e-3 (gate: 2e-2).
  - Per (batch, k-half): 4 PSUM banks hold four 512-wide output stripes; for
    each stripe the 8 filter taps accumulate via matmuls
        psum[k=128p, l<=512] += wp[kh,:,f*128:(f+1)*128].T @ xT[:, l0+f : ...]
  - Eviction fuses bias-add + ReLU (ScalarE activation / VectorE tensor_scalar
    alternating), one merged output DMA per (batch, k-half).
  - Output written k-major ([b, kh, 128, l]); host transposes back to [b, l, k].

Measured (interleaved For_i loop-difference, drift-corrected): 77.9 us per
iteration per core vs 85.4 us for the f32r baseline under the same
methodology. A post-compile BIR pass (_selfload_matmuls) drops the ~254
standalone LDWEIGHTS instructions from the PE queue by flipping the
matmuls to self-loading (-1.0 us). Input DMAs ride the ACT HWDGE queue and output DMAs the SP
queue so next-iteration input prefetch is not serialized behind output
drains (-1.3 us), and the final group's output streams per-stripe to
shorten the tail. The PE matmul stream is the floor: 256 MMs x ~330 ns (213 ns
moving stream + ~120 ns per-MM weight-load serialization that walrus cannot
hide on TRN2; ldw-opt rejects the IR, self-loading MMs measure identically,
and identical consecutive weight APs are still reloaded). fp8e4 DoubleRow
tap-fusion was faster (-6.8 us) but its quantization error (2.1e-2) exceeds
the 2e-2 gate, so it is not used.
"""

import numpy as np

import concourse.bacc as bacc
import concourse.bass as bass
import concourse.tile as tile
import concourse.mybir as mybir
from concourse.bass_utils import run_bass_kernel_spmd

F32 = mybir.dt.float32
BF16 = mybir.dt.bfloat16

N_CORES = 8
B, L, D = 32, 2048, 128
F, K = 8, 256
N_WIN = L - F            # 2040
BP = B // N_CORES        # batches per core
KH = K // 128            # k halves
SUPERS = [(0, 512), (512, 512), (1024, 512), (1536, N_WIN - 1536)]

W_DT = BF16
X_DT = BF16
OUT_DT = BF16


def _selfload_matmuls(nc):
    """Make matmuls self-loading and drop the standalone InstLdweights the
    bass pipeline pairs with them (~254 no-op instructions off the PE queue).
    LDW semaphore waits move onto the paired matmul when it has none, or are
    dropped when implied by a later same-queue DMA wait already on the
    matmul (HWDGE completions within one queue are in-order)."""
    import re

    def qidx(w):
        m = re.match(r"DMAHW(\d+)_", w.ant_name or "")
        return int(m.group(1)) if m else None

    for fn in nc.m.functions:
        for blk in fn.blocks:
            insts = blk.instructions
            keep = []
            for i, x in enumerate(insts):
                tn = type(x).__name__
                if tn == "InstMatmult":
                    x.ldweights = True
                    keep.append(x)
                    continue
                if tn != "InstLdweights":
                    keep.append(x)
                    continue
                si = x.sync_info
                waits = list(si.on_wait) if si else []
                upds = list(si.on_update) if si else []
                if upds:
                    keep.append(x)
                    continue
                if not waits:
                    continue
                nxt = insts[i + 1] if i + 1 < len(insts) else None
                if nxt is None or type(nxt).__name__ != "InstMatmult":
                    keep.append(x)
                    continue
                nsi = nxt.sync_info
                nwaits = list(nsi.on_wait) if nsi else []
                if not nwaits:
                    if nsi is None:
                        nxt.sync_info = mybir.SyncInfo(on_wait=waits,
                                                       on_update=[])
                    else:
                        nsi.on_wait = waits
                    continue
                widx = [qidx(w) for w in waits]
                nidx = [qidx(w) for w in nwaits]
                if (all(v is not None for v in widx)
                        and any(v is not None for v in nidx)
                        and max(widx) < max(v for v in nidx if v is not None)):
                    continue
                keep.append(x)
            blk.instructions[:] = keep


def _build_program(reps=1, loop_n=0):
    """One SPMD program for all 8 cores. loop_n>0 wraps the body in a
    hardware For_i loop (benchmarking only: every iteration rewrites the
    same output region)."""
    nc = bacc.Bacc(
        "TRN2",
        target_bir_lowering=False,
        debug=False,
        num_devices=N_CORES,
    )
    xT_d = nc.declare_dram_parameter("xT", [BP, D, L], X_DT, isOutput=False)
    wp_d = nc.declare_dram_parameter("wp", [KH, D, F * 128], W_DT, isOutput=False)
    bias_d = nc.declare_dram_parameter("bias", [128, KH], F32, isOutput=False)
    out_d = nc.declare_dram_parameter(
        "outT", [reps * BP, KH, 128, N_WIN], OUT_DT, isOutput=True)

    def body(nc, tc, pools, r):
        const_pool, xt_pool, psum_pool, out_pool = pools
        bias_sb = const_pool.tile([128, KH], F32, tag="bias")
        wp_sb = [const_pool.tile([D, F * 128], W_DT, tag=f"wp{kh}",
                                 name=f"wp_sb{kh}")
                 for kh in range(KH)]
        xt = [xt_pool.tile([D, L], X_DT, tag="xt", name=f"xt{b}")
              for b in range(BP)]

        # Inputs go through the ACT HWDGE queue, outputs through the SP
        # queue: two independent hardware DMA queues, so next-iteration
        # input prefetch is not serialized behind output drains.
        # Weights + batch 0 first so compute can start ASAP.
        nc.scalar.dma_start(wp_sb[0][:], wp_d[0])
        nc.scalar.dma_start(xt[0][:], xT_d[0])
        nc.scalar.dma_start(bias_sb[:], bias_d[:])
        nc.scalar.dma_start(wp_sb[1][:], wp_d[1])
        for b in range(1, BP):
            nc.scalar.dma_start(xt[b][:], xT_d[b])

        evictor = 0
        for b in range(BP):
            for kh in range(KH):
                ob = out_pool.tile([128, N_WIN], OUT_DT, tag="ob", name="ob")
                pss = [psum_pool.tile([128, 512], F32, tag="ps",
                                      name=f"ps{b}_{kh}_{si}")
                       for si in range(4)]

                for si in range(4):
                    l0, ls = SUPERS[si]
                    for f in range(F):
                        nc.tensor.matmul(
                            pss[si][:, :ls],
                            lhsT=wp_sb[kh][:, f * 128:(f + 1) * 128],
                            rhs=xt[b][:, l0 + f:l0 + f + ls],
                            start=(f == 0),
                            stop=(f == F - 1),
                        )

                # last group streams per-stripe so the tail is one small DMA
                # after the final eviction instead of one big one
                last = (b == BP - 1 and kh == KH - 1)
                for si in range(4):
                    l0, ls = SUPERS[si]
                    if evictor == 0:
                        nc.scalar.activation(
                            ob[:, l0:l0 + ls], pss[si][:, :ls],
                            mybir.ActivationFunctionType.Relu,
                            bias=bias_sb[:, kh:kh + 1], scale=1.0,
                        )
                    else:
                        nc.vector.tensor_scalar(
                            ob[:, l0:l0 + ls], pss[si][:, :ls],
                            scalar1=bias_sb[:, kh:kh + 1], scalar2=0.0,
                            op0=mybir.AluOpType.add, op1=mybir.AluOpType.max,
                        )
                    evictor ^= 1
                    if last:
                        nc.sync.dma_start(
                            out_d[r * BP + b, kh, :, l0:l0 + ls],
                            ob[:, l0:l0 + ls])
                if not last:
                    eng = nc.sync if (b * KH + kh) % 2 == 0 else nc.scalar
                    eng.dma_start(out_d[r * BP + b, kh], ob[:])

    with tile.TileContext(nc) as tc:
        with (
            tc.tile_pool(name="const", bufs=2) as const_pool,
            tc.tile_pool(name="xt", bufs=BP) as xt_pool,
            tc.tile_pool(name="psum", bufs=8, space=bass.MemorySpace.PSUM) as psum_pool,
            tc.tile_pool(name="out", bufs=6) as out_pool,
        ):
            pools = (const_pool, xt_pool, psum_pool, out_pool)
            if loop_n > 0:
                with tc.For_i(0, loop_n, 1,
                              hint_engines=(mybir.EngineType.PE,)):
                    for r in range(reps):
                        body(nc, tc, pools, r)
            else:
                for r in range(reps):
                    body(nc, tc, pools, r)
    nc.compile()
    _selfload_matmuls(nc)
    return nc


def _prep_inputs(user_batch, filt, W_k, b_k):
    import ml_dtypes
    user_batch = np.asarray(user_batch, dtype=np.float32)
    filt = np.asarray(filt, dtype=np.float32)
    W_k = np.asarray(W_k, dtype=np.float32)
    b_k = np.asarray(b_k, dtype=np.float32)

    wp = W_k.reshape(F, D, K) * filt[:, :, None]              # [f, d, k]
    wp_host = np.ascontiguousarray(                           # [kh, d, f*128]
        wp.reshape(F, D, KH, 128).transpose(2, 1, 0, 3)).reshape(KH, D, F * 128)
    bias_host = np.ascontiguousarray(b_k.reshape(KH, 128).T)  # [128, kh]
    xT = np.ascontiguousarray(user_batch.transpose(0, 2, 1))  # [b, d, l]
    wp_host = wp_host.astype(ml_dtypes.bfloat16)
    xT = xT.astype(ml_dtypes.bfloat16)
    return xT, wp_host, bias_host


def _make_in_maps(xT, wp_host, bias_host):
    return [
        {"xT": xT[c * BP:(c + 1) * BP], "wp": wp_host, "bias": bias_host}
        for c in range(N_CORES)
    ]


def _run(user_batch, filt, W_k, b_k, trace=False):
    xT, wp_host, bias_host = _prep_inputs(user_batch, filt, W_k, b_k)
    nc = _build_program()
    in_maps = _make_in_maps(xT, wp_host, bias_host)
    res = run_bass_kernel_spmd(nc, in_maps, list(range(N_CORES)), trace=trace)
    outT = np.concatenate([np.asarray(r["outT"], dtype=np.float32)
                           for r in res.results], axis=0)  # [B, KH, 128, N_WIN]
    out = outT.reshape(B, K, N_WIN).transpose(0, 2, 1)     # [B, N_WIN, K]
    return np.ascontiguousarray(out), res


def kernel(user_batch, filt, W_k, b_k):
    out, _ = _run(user_batch, filt, W_k, b_k, trace=False)
    return out
